# revision 9
# baseline (speedup 1.0000x reference)
"""Trainium2 Bass kernel for nn_Attention_53231824666818 (GQA attention block).

Sharding: tensor-parallel over heads across 8 NeuronCores. Core c owns query
heads {2c, 2c+1} and kv head c//4 (kv-head groups stay aligned to cores).
Each core computes a full-shape partial of the output projection (row-sharded
Wo); the host sums the 8 partials (fp16 partials, fp32 accumulation).

Device-side layout is fully "transposed": activations live as X^T [hid, seq]
so every matmul contracts over the partition dim with no on-device transposes
of X. Scores are computed as S^T [kpos, qpos], which makes the PV product and
the softmax denominator plain matmuls (ones-matmul broadcast trick) and the
per-query normalization a per-column multiply.

Perf structure (v2):
- All matmul operands are fp16: full PE rate, FWL fast weight loads (fp32r
  gets neither), half the HBM traffic of fp32.
- The only ACT functions used are Exp/Ln/Square/Copy - all in the single
  `natural_log_exp_and_others` table set, so no ACT table reloads.
  RMSNorm rstd = exp(-0.5*ln(ms)); softmax 1/den = exp(-ln(den)). This also
  avoids nc.vector.reciprocal (~4us per [128,512] on DVE).
- Whole X^T is loaded to SBUF up front (8.4 MB fp16) in 4 strip-sized DMAs;
  weights stream in parallel on the other HWDGE ring.
- Three phases, each keeping PE busy back-to-back so the HAM clock gate
  stays at 2.4 GHz: (1) projections + norm/rope epilogues (epilogue chains
  trail one projection target behind PE), (2) flash attention with a
  depth-2 software pipeline (scores matmul for tile t+2 issued before PV of
  tile t, hiding the ACT exp latency), (3) output projection.
- exp uses bias=-EXPC so fp16 probs cannot overflow (softmax shift
  invariance makes this exact).
- V is accumulated directly in [seq, hd] layout (stationary = X^T chunk)
  so no PE transposes are needed.
"""

import math

import numpy as np

import concourse.bass as bass
import concourse.tile as tile
from concourse import mybir

# ---------------------------------------------------------------------------
# Problem constants (hardcoded; kernel.py must be self-contained).
B, S, HID = 1, 2048, 2048
NH, NKV, HD = 16, 2, 128
G = NH // NKV
EPS = 1e-6
THETA = 1000000.0
NCORES = 8
HPC = NH // NCORES          # query heads per core (2)
SW = 512                    # seq strip width
NSTRIP = S // SW            # 4
NHT = HID // 128            # hid-dim k-tiles (16)
NST = S // 128              # seq 128-tiles (16)
ISQ = 1.0 / math.sqrt(HD)
EXPC = 2.0                  # exp bias: keeps fp16 probs < 65504

F32 = mybir.dt.float32
FP16 = mybir.dt.float16

_ALU = mybir.AluOpType
_ACT = mybir.ActivationFunctionType


# ---------------------------------------------------------------------------
# Wait legalization: this walrus build caps fused sync waits at 1 per
# instruction (2 for event-semaphore ops) and rejects any wait on the
# LDWEIGHTS half of a lowered matmul. Tile can attach several waits to one
# instruction (notably the kernel-tail drain), so after TileContext exit we
# hoist excess waits onto same-engine InstNoOp's placed immediately before
# the owner, which blocks the sequencer identically.
_LW_COUNTER = [0]


def _wait_cap(ins) -> int:
    nm = type(ins).__name__
    if nm == "InstMatmult":
        return 0
    if "EventSem" in nm:
        return 2
    return 1


def legalize_waits(nc):
    for fn in nc.m.functions:
        for bb in fn.blocks:
            out = []
            changed = False
            for ins in bb.instructions:
                si = ins.sync_info
                if si is not None:
                    waits = list(si.on_wait or [])
                    cap = _wait_cap(ins)
                    if len(waits) > cap:
                        changed = True
                        for w in waits[cap:]:
                            _LW_COUNTER[0] += 1
                            nop = mybir.InstNoOp(
                                name=f"I-lw-{_LW_COUNTER[0]}",
                                engine=ins.engine,
                                sync_info=mybir.SyncInfo(on_wait=[w], on_update=[]),
                            )
                            out.append(nop)
                        ins.sync_info = mybir.SyncInfo(
                            on_wait=waits[:cap], on_update=list(si.on_update or [])
                        )
                out.append(ins)
            if changed:
                bb.instructions = out
    return nc


# ---------------------------------------------------------------------------
PHASE_MARKS = []


def _mark(nc, label):
    PHASE_MARKS.append((label, int(nc.get_next_instruction_name().split("-")[1])))


def build_nc(legalize=True):
    PHASE_MARKS.clear()
    nc = bass.Bass()

    xtb = nc.dram_tensor("xtb", [128, NSTRIP * NHT * SW], FP16, kind="ExternalInput")
    wqb = nc.dram_tensor("wqb", [128, NHT * HPC * HD], FP16, kind="ExternalInput")
    wkb = nc.dram_tensor("wkb", [128, NHT * HD], FP16, kind="ExternalInput")
    wvb = nc.dram_tensor("wvb", [128, NHT * HD], FP16, kind="ExternalInput")
    wob = nc.dram_tensor("wob", [128, HPC * HID], FP16, kind="ExternalInput")
    # rope tables with the q/k norm weights folded in host-side (the rstd
    # broadcast is constant across partitions, so rmsnorm commutes with rope)
    cosq = nc.dram_tensor("cosq", [HD, S], FP16, kind="ExternalInput")
    sinq = nc.dram_tensor("sinq", [HD, S], FP16, kind="ExternalInput")
    cosk = nc.dram_tensor("cosk", [HD, S], FP16, kind="ExternalInput")
    sink = nc.dram_tensor("sink", [HD, S], FP16, kind="ExternalInput")
    trimask = nc.dram_tensor("trimask", [128, 128], FP16, kind="ExternalInput")
    onesm = nc.dram_tensor("onesm", [128, 128], FP16, kind="ExternalInput")
    epsb = nc.dram_tensor("epsb", [HD, 1], F32, kind="ExternalInput")
    expcb = nc.dram_tensor("expcb", [HD, 1], F32, kind="ExternalInput")
    out = nc.dram_tensor("out", [S, HID], FP16, kind="ExternalOutput")

    with tile.TileContext(nc) as tc:
        with tc.tile_pool(name="persist", bufs=1) as pp, \
             tc.tile_pool(name="epi", bufs=2) as ep, \
             tc.tile_pool(name="exp", bufs=6) as xep, \
             tc.tile_pool(name="obp", bufs=2) as obp, \
             tc.tile_pool(name="ps_acc", bufs=2, space="PSUM") as pacc, \
             tc.tile_pool(name="ps_st", bufs=2, space="PSUM") as pst, \
             tc.tile_pool(name="ps_pv", bufs=2, space="PSUM") as ppv:

            # ---- persistent SBUF residents --------------------------------
            # X strips split into half-tiles so the first projection matmuls
            # gate on a 1 MB DMA instead of the full 2.1 MB strip
            xt_ts = [(pp.tile([128, NHT // 2, SW], FP16, tag=f"xt{s}a", name=f"xt{s}a"),
                      pp.tile([128, NHT // 2, SW], FP16, tag=f"xt{s}b", name=f"xt{s}b"))
                     for s in range(NSTRIP)]
            wq_t = pp.tile([128, NHT, HPC * HD], FP16, tag="wq")
            wk_t = pp.tile([128, NHT, HD], FP16, tag="wk")
            wv_t = pp.tile([128, NHT, HD], FP16, tag="wv")
            wo_t = pp.tile([128, HPC, HID], FP16, tag="wo")
            cosq_t = pp.tile([HD, S], FP16, tag="cosq")
            sinq_t = pp.tile([HD, S], FP16, tag="sinq")
            cosk_t = pp.tile([HD, S], FP16, tag="cosk")
            sink_t = pp.tile([HD, S], FP16, tag="sink")
            eps_t = pp.tile([HD, 1], F32, tag="eps")
            exc_t = pp.tile([HD, 1], F32, tag="exc")
            tri_t = pp.tile([128, 128], FP16, tag="tri")
            ones_t = pp.tile([128, 128], FP16, tag="ones")

            qt0 = pp.tile([HD, S], FP16, tag="qt0")
            qt1 = pp.tile([HD, S], FP16, tag="qt1")
            kt_sb = pp.tile([HD, S], FP16, tag="ktb")
            v_sb = pp.tile([128, NST * HD], FP16, tag="vsb")
            ot0 = pp.tile([HD, S], FP16, tag="ot0")
            ot1 = pp.tile([HD, S], FP16, tag="ot1")

            # ---- input DMAs: ALL on the sync HWDGE ring, in first-use
            # order. Never issue big DMAs from a compute engine: Tile's 8
            # DMAHW completion lanes recycle, so a trigger can block its
            # issuing engine's whole queue behind an earlier DMA (measured
            # 35 us of ACT stall when X strips were triggered from ACT).
            HSW = (NHT // 2) * SW
            nc.sync.dma_start(xt_ts[0][0][:], xtb[:, 0:HSW])
            nc.sync.dma_start(wq_t[:], wqb[:])
            nc.sync.dma_start(xt_ts[0][1][:], xtb[:, HSW:2 * HSW])
            nc.sync.dma_start(wk_t[:], wkb[:])
            nc.sync.dma_start(wv_t[:], wvb[:])
            nc.sync.dma_start(xt_ts[1][0][:], xtb[:, 2 * HSW:3 * HSW])
            nc.sync.dma_start(xt_ts[1][1][:], xtb[:, 3 * HSW:4 * HSW])
            nc.sync.dma_start(cosq_t[:], cosq[:])
            nc.sync.dma_start(sinq_t[:], sinq[:])
            nc.sync.dma_start(cosk_t[:], cosk[:])
            nc.sync.dma_start(sink_t[:], sink[:])
            for t, d in ((eps_t, epsb),
                         (exc_t, expcb), (tri_t, trimask), (ones_t, onesm)):
                nc.sync.dma_start(t[:], d[:])
            for s in (2, 3):
                nc.sync.dma_start(xt_ts[s][0][:], xtb[:, 2 * s * HSW:(2 * s + 1) * HSW])
                nc.sync.dma_start(xt_ts[s][1][:], xtb[:, (2 * s + 1) * HSW:(2 * s + 2) * HSW])
            nc.sync.dma_start(wo_t[:], wob[:])

            # ---- PE warmup: the HAM clock gate needs ~3.4us of sustained
            # matmul activity to lift PE from 1.2 to 2.4 GHz. The first real
            # matmul can only start once xt0a+wq land (~15us); run throwaway
            # matmuls on a zeroed tile during the DMA window so the real work
            # starts at full clock.
            warm_src = ep.tile([128, SW], FP16, tag="warm", bufs=1)
            nc.vector.memset(warm_src[:], 0.0)
            warm_ps = pst.tile([128, 2, SW], F32, tag="st")
            for i in range(55):
                nc.tensor.matmul(warm_ps[:, 0, :], warm_src[:, 0:128],
                                 warm_src[:], start=True, stop=True)

            # ---- norm + rope epilogue, split in two so the ssq matmul never
            # stalls PE: partA (engine-only drains of the PSUM acc) is emitted
            # right after the projection; chainB one projection target later.
            def epi_partA(acc):
                sq = ep.tile([128, SW], FP16, tag="sq")
                nc.scalar.activation(sq[:], acc[:], _ACT.Square)
                qc = ep.tile([128, SW], FP16, tag="qc")
                nc.scalar.copy(qc[:], acc[:])
                return sq, qc

            def epi_chainB(sq, qc, ctab, stab, dst, sl):
                ssq = pst.tile([128, SW], F32, tag="st")
                nc.tensor.matmul(ssq[:], ones_t[:], sq[:], start=True, stop=True)
                # rstd = (ms)^-1/2 = exp(-0.5*ln(ssq/HD + eps)); Ln/Exp live in
                # one ACT table set (sqrt does not, and DVE reciprocal is slow)
                lnv = ep.tile([128, SW], F32, tag="lnv")
                nc.scalar.activation(lnv[:], ssq[:], _ACT.Ln,
                                     scale=1.0 / HD, bias=eps_t[:])
                rstd = ep.tile([128, SW], FP16, tag="rstd")
                nc.scalar.activation(rstd[:], lnv[:], _ACT.Exp, scale=-0.5)
                # rope entirely in fp16: every tensor_tensor runs in the DVE
                # 2x packed mode (rmsnorm scale applied after rotation - the
                # rstd broadcast is per-column so it commutes)
                t1 = ep.tile([128, SW], FP16, tag="t1")
                nc.vector.tensor_tensor(out=t1[:], in0=qc[:], in1=ctab[:, sl],
                                        op=_ALU.mult)
                u = ep.tile([128, SW], FP16, tag="u")
                nc.vector.tensor_tensor(out=u[0:64, :], in0=qc[64:128, :],
                                        in1=stab[64:128, sl], op=_ALU.mult)
                nc.vector.tensor_tensor(out=u[64:128, :], in0=qc[0:64, :],
                                        in1=stab[0:64, sl], op=_ALU.mult)
                pre = ep.tile([128, SW], FP16, tag="pre")
                nc.vector.tensor_tensor(out=pre[:], in0=t1[:], in1=u[:],
                                        op=_ALU.add)
                nc.vector.tensor_tensor(out=dst[:, sl], in0=pre[:],
                                        in1=rstd[:], op=_ALU.mult)

            # ---- phase 1: projections ------------------------------------
            # epi chainB's trail TWO projection targets behind, so the ssq
            # matmul never waits on the ACT-engine Square latency.
            pending = []

            def push_chain(item=None):
                if len(pending) > (1 if item is not None else 0):
                    epi_chainB(*pending.pop(0))
                if item is not None:
                    pending.append(item)

            def phase_a(s):
                sl = bass.ts(s, SW)
                _mark(nc, f"A{s}")
                xta, xtb_ = xt_ts[s]
                xsel = lambda h: (xta if h < NHT // 2 else xtb_)[:, h % (NHT // 2), :]
                targets = (
                    (lambda h: wq_t[:, h, 0:HD], cosq_t, sinq_t, qt0),
                    (lambda h: wq_t[:, h, HD:2 * HD], cosq_t, sinq_t, qt1),
                    (lambda h: wk_t[:, h, :], cosk_t, sink_t, kt_sb),
                )
                for wsel, ctab, stab, dst in targets:
                    acc = pacc.tile([128, SW], F32, tag="acc")
                    for h in range(NHT):
                        nc.tensor.matmul(acc[:], wsel(h), xsel(h),
                                         start=(h == 0), stop=(h == NHT - 1))
                    pa = epi_partA(acc)
                    push_chain((pa[0], pa[1], ctab, stab, dst, sl))
                # V: accumulate directly in [seq, hd] (stationary = X^T
                # chunk); two 128-seq chunks share one PSUM bank and drain
                # with a single DVE copy
                for jp in range(2):
                    vacc = pacc.tile([128, 2, HD], F32, tag="acc")
                    for half in range(2):
                        j = 2 * jp + half
                        for h in range(NHT):
                            xh = (xta if h < NHT // 2 else xtb_)
                            nc.tensor.matmul(vacc[:, half, :],
                                             xh[:, h % (NHT // 2), bass.ts(j, 128)],
                                             wv_t[:, h, :],
                                             start=(h == 0), stop=(h == NHT - 1))
                    nc.vector.tensor_copy(
                        v_sb[:, bass.ts(2 * s + jp, 256)], vacc[:, :, :])

            # ---- phase 2: attention, software-pipelined in PAIRS of key
            # tiles. Scores for a pair land in one 2-bank PSUM tile so a
            # single wide ACT exp covers both tiles ((N+352) fixed overhead
            # amortizes: 574 ns/tile vs 720, below the 645 ns/tile PE rate).
            def attention(qt, ot, s):
                sl = bass.ts(s, SW)
                nk = 4 * s + 4
                pv = ppv.tile([128, SW], F32, tag="pv")
                den = ppv.tile([128, SW], F32, tag="pv")
                exs = {}

                def emit_pair(pi):
                    t0 = 2 * pi
                    stp = pst.tile([128, 2, SW], F32, tag="st")
                    exp_ = xep.tile([128, 2, SW], FP16, tag="ex")
                    vss = []
                    for half, t in enumerate((t0, t0 + 1)):
                        off = t - 4 * s
                        vs = 0 if off < 0 else 128 * off
                        vss.append(vs)
                        nc.tensor.matmul(stp[:, half, vs:],
                                         kt_sb[:, bass.ts(t, 128)],
                                         qt[:, SW * s + vs:SW * (s + 1)],
                                         start=True, stop=True)
                    if t0 + 1 < 4 * s:
                        # both halves full: one wide exp
                        nc.scalar.activation(exp_[:, :, :], stp[:, :, :],
                                             _ACT.Exp, scale=ISQ,
                                             bias=exc_t[:])
                    else:
                        for half, vs in enumerate(vss):
                            nc.scalar.activation(exp_[:, half, vs:],
                                                 stp[:, half, vs:], _ACT.Exp,
                                                 scale=ISQ, bias=exc_t[:])
                            # diagonal 128x128 block needs the triangular mask
                            nc.vector.tensor_tensor(
                                out=exp_[:, half, vs:vs + 128],
                                in0=exp_[:, half, vs:vs + 128],
                                in1=tri_t[:], op=_ALU.mult)
                    exs[pi] = (exp_, vss)

                emit_pair(0)
                for pi in range(nk // 2):
                    if pi + 1 < nk // 2:
                        emit_pair(pi + 1)
                    exp_, vss = exs.pop(pi)
                    for half, vs in enumerate(vss):
                        t = 2 * pi + half
                        st_, sp_ = (t == 0), (t == nk - 1)
                        nc.tensor.matmul(pv[:, vs:], v_sb[:, bass.ts(t, 128)],
                                         exp_[:, half, vs:], start=st_, stop=sp_)
                        nc.tensor.matmul(den[:, vs:], ones_t[:],
                                         exp_[:, half, vs:], start=st_, stop=sp_)
                # 1/den = exp(-ln(den)) on ACT (same table set as the exps)
                lnd = ep.tile([128, SW], F32, tag="lnd")
                nc.scalar.activation(lnd[:], den[:], _ACT.Ln)
                rdn = ep.tile([128, SW], F32, tag="rdn")
                nc.scalar.activation(rdn[:], lnd[:], _ACT.Exp, scale=-1.0)
                nc.vector.tensor_tensor(out=ot[:, sl], in0=pv[:], in1=rdn[:],
                                        op=_ALU.mult)

            # ---- phase 3: output projection (per 128-row tile) ------------
            def outproj(cs):
                _mark(nc, f"C{cs}")
                for m in range(4 * cs, 4 * cs + 4):
                    ob = obp.tile([128, HID], FP16, tag="ob")
                    for np_ in range(2):
                        oup = pst.tile([128, 2, SW], F32, tag="st")
                        for half in range(2):
                            n = 2 * np_ + half
                            nc.tensor.matmul(oup[:, half, :],
                                             ot0[:, bass.ts(m, 128)],
                                             wo_t[:, 0, bass.ts(n, SW)],
                                             start=True, stop=False)
                            nc.tensor.matmul(oup[:, half, :],
                                             ot1[:, bass.ts(m, 128)],
                                             wo_t[:, 1, bass.ts(n, SW)],
                                             start=False, stop=True)
                        # alternate the wide pair copies between ACT and DVE
                        # so neither engine bottlenecks the outproj stream
                        if (m + np_) % 2:
                            nc.scalar.copy(ob[:, bass.ts(np_, 2 * SW)],
                                           oup[:, :, :])
                        else:
                            nc.vector.tensor_copy(ob[:, bass.ts(np_, 2 * SW)],
                                                  oup[:, :, :])
                    nc.sync.dma_start(out[bass.ts(m, 128), :], ob[:])

            # schedule: A0 A1 B(s0) A2 B(s1) C0 A3 B(s2) C1 B(s3) C2 C3
            # (attention for strip s slots in right after A(s+1), keeping PE
            # dense across phase boundaries and the HAM clock warm)
            def phase_b(s):
                _mark(nc, f"B0s{s}")
                attention(qt0, ot0, s)
                _mark(nc, f"B1s{s}")
                attention(qt1, ot1, s)

            phase_a(0)
            phase_a(1)
            phase_b(0)
            phase_a(2)
            phase_b(1)
            outproj(0)
            phase_a(3)
            push_chain()
            push_chain()
            phase_b(2)
            outproj(1)
            phase_b(3)
            outproj(2)
            outproj(3)

    if legalize:
        legalize_waits(nc)
    return nc


# ---------------------------------------------------------------------------
# Host-side input prep.
def _rope_tables(position_ids: np.ndarray):
    pos = position_ids.reshape(-1).astype(np.float64)  # [S]
    j = np.arange(0, HD, 2, dtype=np.float64)
    inv_freq = 1.0 / (THETA ** (j / HD))               # [HD/2]
    freqs = np.outer(inv_freq, pos)                    # [HD/2, S]
    cos_h = np.cos(freqs)
    sin_h = np.sin(freqs)
    cosT = np.concatenate([cos_h, cos_h], axis=0).astype(np.float32)
    sinN = np.concatenate([sin_h, -sin_h], axis=0).astype(np.float32)
    return np.ascontiguousarray(cosT), np.ascontiguousarray(sinN)


def _prep_in_maps(hidden_states, Wq, Wk, Wv, Wo, q_norm_w, k_norm_w,
                  position_ids):
    X = np.asarray(hidden_states, dtype=np.float32).reshape(S, HID)
    # xtb[p, s, h, w] = X[s*SW+w, h*128+p]
    xtb = np.ascontiguousarray(
        X.reshape(NSTRIP, SW, NHT, 128).transpose(3, 0, 2, 1)
        .reshape(128, NSTRIP * NHT * SW)).astype(np.float16)
    cosT, sinN = _rope_tables(np.asarray(position_ids))
    qw = np.asarray(q_norm_w, dtype=np.float32).reshape(HD, 1)
    kw = np.asarray(k_norm_w, dtype=np.float32).reshape(HD, 1)
    cosq = np.ascontiguousarray(cosT * qw).astype(np.float16)
    sinq = np.ascontiguousarray(sinN * qw).astype(np.float16)
    cosk = np.ascontiguousarray(cosT * kw).astype(np.float16)
    sink = np.ascontiguousarray(sinN * kw).astype(np.float16)
    kp, qp = np.meshgrid(np.arange(128), np.arange(128), indexing="ij")
    trimask = (qp >= kp).astype(np.float16)
    onesm = np.ones((128, 128), np.float16)

    Wq = np.asarray(Wq, dtype=np.float32)
    Wk = np.asarray(Wk, dtype=np.float32)
    Wv = np.asarray(Wv, dtype=np.float32)
    Wo = np.asarray(Wo, dtype=np.float32)

    in_maps = []
    for c in range(NCORES):
        kv = c // (NCORES // NKV)
        wq_c = Wq[:, c * HPC * HD:(c + 1) * HPC * HD]
        wq_l = np.ascontiguousarray(
            wq_c.reshape(NHT, 128, HPC * HD).transpose(1, 0, 2).reshape(
                128, NHT * HPC * HD)).astype(np.float16)
        wk_c = Wk[:, kv * HD:(kv + 1) * HD]
        wk_l = np.ascontiguousarray(
            wk_c.reshape(NHT, 128, HD).transpose(1, 0, 2).reshape(
                128, NHT * HD)).astype(np.float16)
        wv_c = Wv[:, kv * HD:(kv + 1) * HD]
        wv_l = np.ascontiguousarray(
            wv_c.reshape(NHT, 128, HD).transpose(1, 0, 2).reshape(
                128, NHT * HD)).astype(np.float16)
        wo_c = Wo[c * HPC * HD:(c + 1) * HPC * HD, :]
        wo_l = np.ascontiguousarray(
            wo_c.reshape(HPC, HD, HID).transpose(1, 0, 2).reshape(
                128, HPC * HID)).astype(np.float16)
        in_maps.append({
            "xtb": xtb, "wqb": wq_l, "wkb": wk_l, "wvb": wv_l, "wob": wo_l,
            "cosq": cosq, "sinq": sinq, "cosk": cosk, "sink": sink,
            "trimask": trimask, "onesm": onesm,
            "epsb": np.full((HD, 1), EPS, np.float32),
            "expcb": np.full((HD, 1), -EXPC, np.float32),
        })
    return in_maps


# ---------------------------------------------------------------------------
# Runner: persistent jitted shard_map over 8 cores (no donation so device
# buffers are reusable across timing iterations).
_CACHE: dict = {}


def _make_runner(nc):
    import jax
    from jax.sharding import Mesh, PartitionSpec
    try:
        from jax.experimental.shard_map import shard_map
    except ImportError:
        from jax.shard_map import shard_map
    from concourse.bass2jax import (_bass_exec_p, install_neuronx_cc_hook,
                                    partition_id_tensor)

    install_neuronx_cc_hook()

    partition_name = (nc.partition_id_tensor.name
                      if nc.partition_id_tensor else None)
    in_names, out_names, out_avals, zero_outs = [], [], [], []
    for alloc in nc.m.functions[0].allocations:
        if not isinstance(alloc, mybir.MemoryLocationSet):
            continue
        name = alloc.memorylocations[0].name
        if alloc.kind == "ExternalInput":
            if name != partition_name:
                in_names.append(name)
        elif alloc.kind == "ExternalOutput":
            shape = list(alloc.tensor_shape)
            npdt = mybir.dt.np(alloc.dtype)
            out_names.append(name)
            out_avals.append(jax.core.ShapedArray(shape, npdt))
            zero_outs.append(np.zeros(shape, npdt))

    n_params = len(in_names)
    all_in_names = list(in_names) + list(out_names)
    if partition_name is not None:
        all_in_names.append(partition_name)

    def _body(*args):
        operands = list(args)
        if partition_name is not None:
            operands.append(partition_id_tensor())
        outs = _bass_exec_p.bind(
            *operands,
            out_avals=tuple(out_avals),
            in_names=tuple(all_in_names),
            out_names=tuple(out_names),
            lowering_input_output_aliases=(),
            sim_require_finite=True,
            sim_require_nnan=True,
            nc=nc,
        )
        return tuple(outs)

    devices = jax.devices()[:NCORES]
    mesh = Mesh(np.asarray(devices), ("core",))
    n_outs = len(out_names)
    sharded = jax.jit(
        shard_map(_body, mesh=mesh,
                  in_specs=(PartitionSpec("core"),) * (n_params + n_outs),
                  out_specs=(PartitionSpec("core"),) * n_outs,
                  check_rep=False),
        keep_unused=True,
    )
    return {
        "fn": sharded, "in_names": in_names, "out_names": out_names,
        "out_avals": out_avals, "zero_outs": zero_outs, "jax": jax,
    }


def _get_runner(which="main"):
    key = f"runner_{which}"
    if key not in _CACHE:
        nc = build_nc() if which == "main" else build_null_nc()
        _CACHE[f"nc_{which}"] = nc
        _CACHE[key] = _make_runner(nc)
    return _CACHE[key]


def _device_args(in_maps, which="main"):
    r = _get_runner(which)
    jax = r["jax"]
    concat_in = [
        np.concatenate([np.asarray(in_maps[c][name]) for c in range(NCORES)],
                       axis=0)
        for name in r["in_names"]
    ]
    concat_zeros = [
        np.zeros((NCORES * z.shape[0], *z.shape[1:]), z.dtype)
        for z in r["zero_outs"]
    ]
    return [jax.device_put(a) for a in (concat_in + concat_zeros)]


def _run(dargs, which="main"):
    r = _get_runner(which)
    outs = r["fn"](*dargs)
    return outs


def kernel(**inputs) -> np.ndarray:
    in_maps = _prep_in_maps(**inputs)
    dargs = _device_args(in_maps)
    outs = _run(dargs)
    out_c = np.asarray(outs[0]).reshape(NCORES, S, HID)
    full = out_c.astype(np.float32).sum(axis=0).astype(np.float32)
    return full.reshape(B, S, HID)


def build_null_nc(legalize=True):
    """Input-identical null kernel: same ExternalInput/Output set, but only a
    trivial copy. Used to calibrate away per-dispatch input-staging overhead
    when estimating device execution time."""
    nc = bass.Bass()
    tensors = [
        ("xtb", [128, NSTRIP * NHT * SW], FP16),
        ("wqb", [128, NHT * HPC * HD], FP16),
        ("wkb", [128, NHT * HD], FP16), ("wvb", [128, NHT * HD], FP16),
        ("wob", [128, HPC * HID], FP16), ("cosq", [HD, S], FP16),
        ("sinq", [HD, S], FP16), ("cosk", [HD, S], FP16),
        ("sink", [HD, S], FP16),
        ("trimask", [128, 128], FP16), ("onesm", [128, 128], FP16),
        ("epsb", [HD, 1], F32), ("expcb", [HD, 1], F32),
    ]
    handles = {}
    for name, shape, dt in tensors:
        handles[name] = nc.dram_tensor(name, shape, dt, kind="ExternalInput")
    out = nc.dram_tensor("out", [S, HID], FP16, kind="ExternalOutput")
    with tile.TileContext(nc) as tc:
        with tc.tile_pool(name="sb", bufs=1) as sb:
            t = sb.tile([128, 128], FP16)
            nc.sync.dma_start(t[:], handles["trimask"][:])
            nc.sync.dma_start(out[0:128, 0:128], t[:])
    if legalize:
        legalize_waits(nc)
    return nc


def timed_run(inputs, iters=60):
    """Estimate on-device execution time.

    Per-call wall time through the axon tunnel is dominated by input staging
    (~30 ms for this input set), so we interleave single calls of the real
    kernel and an input-identical null kernel and difference the medians of
    the paired per-call times."""
    import time
    in_maps = _prep_in_maps(**inputs)
    d_main = _device_args(in_maps, "main")
    d_null = _device_args(in_maps, "null")
    r_main = _get_runner("main")
    r_null = _get_runner("null")
    jax = r_main["jax"]
    jax.block_until_ready(_run(d_main, "main"))
    jax.block_until_ready(_run(d_null, "null"))

    tm, tn = [], []
    for _ in range(iters):
        t0 = time.perf_counter()
        jax.block_until_ready(_run(d_null, "null"))
        tn.append(time.perf_counter() - t0)
        t0 = time.perf_counter()
        jax.block_until_ready(_run(d_main, "main"))
        tm.append(time.perf_counter() - t0)
    tm, tn = np.array(tm), np.array(tn)
    est = float(np.median(tm) - np.median(tn))
    return max(est, 0.0), float(np.median(tm)), float(np.median(tn))


# revision 10
# speedup vs baseline: 1.1525x; 1.1525x over previous
"""Trainium2 Bass kernel for nn_Attention_53231824666818 (GQA attention block).

Sharding: tensor-parallel over heads across 8 NeuronCores. Core c owns query
heads {2c, 2c+1} and kv head c//4 (kv-head groups stay aligned to cores).
Each core computes a full-shape partial of the output projection (row-sharded
Wo); the host sums the 8 partials (fp16 partials, fp32 accumulation).

Device-side layout is fully "transposed": activations live as X^T [hid, seq]
so every matmul contracts over the partition dim with no on-device transposes
of X. Scores are computed as S^T [kpos, qpos], which makes the PV product and
the softmax denominator plain matmuls (ones-matmul broadcast trick) and the
per-query normalization a per-column multiply.

Perf structure (v2):
- All matmul operands are fp16: full PE rate, FWL fast weight loads (fp32r
  gets neither), half the HBM traffic of fp32.
- The only ACT functions used are Exp/Ln/Square/Copy - all in the single
  `natural_log_exp_and_others` table set, so no ACT table reloads.
  RMSNorm rstd = exp(-0.5*ln(ms)); softmax 1/den = exp(-ln(den)). This also
  avoids nc.vector.reciprocal (~4us per [128,512] on DVE).
- Whole X^T is loaded to SBUF up front (8.4 MB fp16) in 4 strip-sized DMAs;
  weights stream in parallel on the other HWDGE ring.
- Three phases, each keeping PE busy back-to-back so the HAM clock gate
  stays at 2.4 GHz: (1) projections + norm/rope epilogues (epilogue chains
  trail one projection target behind PE), (2) flash attention with a
  depth-2 software pipeline (scores matmul for tile t+2 issued before PV of
  tile t, hiding the ACT exp latency), (3) output projection.
- exp uses bias=-EXPC so fp16 probs cannot overflow (softmax shift
  invariance makes this exact).
- V is accumulated directly in [seq, hd] layout (stationary = X^T chunk)
  so no PE transposes are needed.
"""

import math

import numpy as np

import concourse.bass as bass
import concourse.tile as tile
from concourse import mybir

# ---------------------------------------------------------------------------
# Problem constants (hardcoded; kernel.py must be self-contained).
B, S, HID = 1, 2048, 2048
NH, NKV, HD = 16, 2, 128
G = NH // NKV
EPS = 1e-6
THETA = 1000000.0
NCORES = 8
HPC = NH // NCORES          # query heads per core (2)
SW = 512                    # seq strip width
NSTRIP = S // SW            # 4
NHT = HID // 128            # hid-dim k-tiles (16)
NST = S // 128              # seq 128-tiles (16)
ISQ = 1.0 / math.sqrt(HD)
EXPC = 2.0                  # exp bias: keeps fp16 probs < 65504

F32 = mybir.dt.float32
FP16 = mybir.dt.float16

_ALU = mybir.AluOpType
_ACT = mybir.ActivationFunctionType


# ---------------------------------------------------------------------------
# Wait legalization: this walrus build caps fused sync waits at 1 per
# instruction (2 for event-semaphore ops) and rejects any wait on the
# LDWEIGHTS half of a lowered matmul. Tile can attach several waits to one
# instruction (notably the kernel-tail drain), so after TileContext exit we
# hoist excess waits onto same-engine InstNoOp's placed immediately before
# the owner, which blocks the sequencer identically.
_LW_COUNTER = [0]


def _wait_cap(ins) -> int:
    nm = type(ins).__name__
    if nm == "InstMatmult":
        return 0
    if "EventSem" in nm:
        return 2
    return 1


def legalize_waits(nc):
    for fn in nc.m.functions:
        for bb in fn.blocks:
            out = []
            changed = False
            for ins in bb.instructions:
                si = ins.sync_info
                if si is not None:
                    waits = list(si.on_wait or [])
                    cap = _wait_cap(ins)
                    if len(waits) > cap:
                        changed = True
                        for w in waits[cap:]:
                            _LW_COUNTER[0] += 1
                            nop = mybir.InstNoOp(
                                name=f"I-lw-{_LW_COUNTER[0]}",
                                engine=ins.engine,
                                sync_info=mybir.SyncInfo(on_wait=[w], on_update=[]),
                            )
                            out.append(nop)
                        ins.sync_info = mybir.SyncInfo(
                            on_wait=waits[:cap], on_update=list(si.on_update or [])
                        )
                out.append(ins)
            if changed:
                bb.instructions = out
    return nc


# ---------------------------------------------------------------------------
PHASE_MARKS = []


def _mark(nc, label):
    PHASE_MARKS.append((label, int(nc.get_next_instruction_name().split("-")[1])))


def build_nc(legalize=True):
    PHASE_MARKS.clear()
    nc = bass.Bass()

    xtb = nc.dram_tensor("xtb", [128, NSTRIP * NHT * SW], FP16, kind="ExternalInput")
    wqb = nc.dram_tensor("wqb", [128, NHT * HPC * HD], FP16, kind="ExternalInput")
    wkb = nc.dram_tensor("wkb", [128, NHT * HD], FP16, kind="ExternalInput")
    wvb = nc.dram_tensor("wvb", [128, NHT * HD], FP16, kind="ExternalInput")
    wob = nc.dram_tensor("wob", [128, HPC * HID], FP16, kind="ExternalInput")
    # rope tables with the q/k norm weights folded in host-side (the rstd
    # broadcast is constant across partitions, so rmsnorm commutes with rope)
    cosq = nc.dram_tensor("cosq", [HD, S], FP16, kind="ExternalInput")
    sinq = nc.dram_tensor("sinq", [HD, S], FP16, kind="ExternalInput")
    cosk = nc.dram_tensor("cosk", [HD, S], FP16, kind="ExternalInput")
    sink = nc.dram_tensor("sink", [HD, S], FP16, kind="ExternalInput")
    trimask = nc.dram_tensor("trimask", [128, 128], FP16, kind="ExternalInput")
    onesm = nc.dram_tensor("onesm", [128, 128], FP16, kind="ExternalInput")
    epsb = nc.dram_tensor("epsb", [HD, 1], F32, kind="ExternalInput")
    expcb = nc.dram_tensor("expcb", [HD, 1], F32, kind="ExternalInput")
    out = nc.dram_tensor("out", [S, HID], FP16, kind="ExternalOutput")

    with tile.TileContext(nc) as tc:
        with tc.tile_pool(name="persist", bufs=1) as pp, \
             tc.tile_pool(name="epi", bufs=2) as ep, \
             tc.tile_pool(name="exp", bufs=6) as xep, \
             tc.tile_pool(name="obp", bufs=2) as obp, \
             tc.tile_pool(name="ps_acc", bufs=2, space="PSUM") as pacc, \
             tc.tile_pool(name="ps_st", bufs=2, space="PSUM") as pst, \
             tc.tile_pool(name="ps_pv", bufs=2, space="PSUM") as ppv:

            # ---- persistent SBUF residents --------------------------------
            # X strips split into half-tiles so the first projection matmuls
            # gate on a 1 MB DMA instead of the full 2.1 MB strip
            xt_ts = [(pp.tile([128, NHT // 2, SW], FP16, tag=f"xt{s}a", name=f"xt{s}a"),
                      pp.tile([128, NHT // 2, SW], FP16, tag=f"xt{s}b", name=f"xt{s}b"))
                     for s in range(NSTRIP)]
            wq_t = pp.tile([128, NHT, HPC * HD], FP16, tag="wq")
            wk_t = pp.tile([128, NHT, HD], FP16, tag="wk")
            wv_t = pp.tile([128, NHT, HD], FP16, tag="wv")
            wo_t = pp.tile([128, HPC, HID], FP16, tag="wo")
            cosq_t = pp.tile([HD, S], FP16, tag="cosq")
            sinq_t = pp.tile([HD, S], FP16, tag="sinq")
            cosk_t = pp.tile([HD, S], FP16, tag="cosk")
            sink_t = pp.tile([HD, S], FP16, tag="sink")
            eps_t = pp.tile([HD, 1], F32, tag="eps")
            exc_t = pp.tile([HD, 1], F32, tag="exc")
            tri_t = pp.tile([128, 128], FP16, tag="tri")
            ones_t = pp.tile([128, 128], FP16, tag="ones")

            qt0 = pp.tile([HD, S], FP16, tag="qt0")
            qt1 = pp.tile([HD, S], FP16, tag="qt1")
            kt_sb = pp.tile([HD, S], FP16, tag="ktb")
            v_sb = pp.tile([128, NST * HD], FP16, tag="vsb")
            ot0 = pp.tile([HD, S], FP16, tag="ot0")
            ot1 = pp.tile([HD, S], FP16, tag="ot1")

            # ---- input DMAs: ALL on the sync HWDGE ring, in first-use
            # order. Never issue big DMAs from a compute engine: Tile's 8
            # DMAHW completion lanes recycle, so a trigger can block its
            # issuing engine's whole queue behind an earlier DMA (measured
            # 35 us of ACT stall when X strips were triggered from ACT).
            HSW = (NHT // 2) * SW
            nc.sync.dma_start(xt_ts[0][0][:], xtb[:, 0:HSW])
            nc.sync.dma_start(wq_t[:], wqb[:])
            nc.sync.dma_start(xt_ts[0][1][:], xtb[:, HSW:2 * HSW])
            nc.sync.dma_start(wk_t[:], wkb[:])
            nc.sync.dma_start(wv_t[:], wvb[:])
            nc.sync.dma_start(xt_ts[1][0][:], xtb[:, 2 * HSW:3 * HSW])
            nc.sync.dma_start(xt_ts[1][1][:], xtb[:, 3 * HSW:4 * HSW])
            nc.sync.dma_start(cosq_t[:], cosq[:])
            nc.sync.dma_start(sinq_t[:], sinq[:])
            nc.sync.dma_start(cosk_t[:], cosk[:])
            nc.sync.dma_start(sink_t[:], sink[:])
            for t, d in ((eps_t, epsb),
                         (exc_t, expcb), (tri_t, trimask), (ones_t, onesm)):
                nc.sync.dma_start(t[:], d[:])
            for s in (2, 3):
                nc.sync.dma_start(xt_ts[s][0][:], xtb[:, 2 * s * HSW:(2 * s + 1) * HSW])
                nc.sync.dma_start(xt_ts[s][1][:], xtb[:, (2 * s + 1) * HSW:(2 * s + 2) * HSW])
            nc.sync.dma_start(wo_t[:], wob[:])

            # ---- PE warmup: the HAM clock gate needs ~3.4us of sustained
            # matmul activity to lift PE from 1.2 to 2.4 GHz. The first real
            # matmul can only start once xt0a+wq land (~15us); run throwaway
            # matmuls on a zeroed tile during the DMA window so the real work
            # starts at full clock.
            warm_src = ep.tile([128, SW], FP16, tag="warm", bufs=1)
            nc.vector.memset(warm_src[:], 0.0)
            warm_ps = pst.tile([128, 2, SW], F32, tag="st")
            for i in range(55):
                nc.tensor.matmul(warm_ps[:, 0, :], warm_src[:, 0:128],
                                 warm_src[:], start=True, stop=True)

            # ---- norm + rope epilogue, split in two so the ssq matmul never
            # stalls PE: partA (engine-only drains of the PSUM acc) is emitted
            # right after the projection; chainB one projection target later.
            def epi_partA(acc):
                sq = ep.tile([128, SW], FP16, tag="sq")
                nc.scalar.activation(sq[:], acc[:], _ACT.Square)
                qc = ep.tile([128, SW], FP16, tag="qc")
                nc.vector.tensor_copy(qc[:], acc[:])
                return sq, qc

            def epi_chainB(sq, qc, ctab, stab, dst, sl):
                ssq = pst.tile([128, SW], F32, tag="st")
                nc.tensor.matmul(ssq[:], ones_t[:], sq[:], start=True, stop=True)
                # rstd = (ms)^-1/2 = exp(-0.5*ln(ssq/HD + eps)); Ln/Exp live in
                # one ACT table set (sqrt does not, and DVE reciprocal is slow)
                lnv = ep.tile([128, SW], F32, tag="lnv")
                nc.scalar.activation(lnv[:], ssq[:], _ACT.Ln,
                                     scale=1.0 / HD, bias=eps_t[:])
                rstd = ep.tile([128, SW], FP16, tag="rstd")
                nc.scalar.activation(rstd[:], lnv[:], _ACT.Exp, scale=-0.5)
                # rope entirely in fp16: every tensor_tensor runs in the DVE
                # 2x packed mode (rmsnorm scale applied after rotation - the
                # rstd broadcast is per-column so it commutes)
                t1 = ep.tile([128, SW], FP16, tag="t1")
                nc.vector.tensor_tensor(out=t1[:], in0=qc[:], in1=ctab[:, sl],
                                        op=_ALU.mult)
                u = ep.tile([128, SW], FP16, tag="u")
                nc.vector.tensor_tensor(out=u[0:64, :], in0=qc[64:128, :],
                                        in1=stab[64:128, sl], op=_ALU.mult)
                nc.vector.tensor_tensor(out=u[64:128, :], in0=qc[0:64, :],
                                        in1=stab[0:64, sl], op=_ALU.mult)
                pre = ep.tile([128, SW], FP16, tag="pre")
                nc.vector.tensor_tensor(out=pre[:], in0=t1[:], in1=u[:],
                                        op=_ALU.add)
                nc.vector.tensor_tensor(out=dst[:, sl], in0=pre[:],
                                        in1=rstd[:], op=_ALU.mult)

            # ---- phase 1: projections ------------------------------------
            # epi chainB's trail TWO projection targets behind, so the ssq
            # matmul never waits on the ACT-engine Square latency.
            pending = []

            def push_chain(item=None):
                if len(pending) > (1 if item is not None else 0):
                    epi_chainB(*pending.pop(0))
                if item is not None:
                    pending.append(item)

            def phase_a(s):
                sl = bass.ts(s, SW)
                _mark(nc, f"A{s}")
                xta, xtb_ = xt_ts[s]
                xsel = lambda h: (xta if h < NHT // 2 else xtb_)[:, h % (NHT // 2), :]
                targets = (
                    (lambda h: wq_t[:, h, 0:HD], cosq_t, sinq_t, qt0),
                    (lambda h: wq_t[:, h, HD:2 * HD], cosq_t, sinq_t, qt1),
                    (lambda h: wk_t[:, h, :], cosk_t, sink_t, kt_sb),
                )
                for wsel, ctab, stab, dst in targets:
                    acc = pacc.tile([128, SW], F32, tag="acc")
                    for h in range(NHT):
                        nc.tensor.matmul(acc[:], wsel(h), xsel(h),
                                         start=(h == 0), stop=(h == NHT - 1))
                    pa = epi_partA(acc)
                    push_chain((pa[0], pa[1], ctab, stab, dst, sl))
                # V: accumulate directly in [seq, hd] (stationary = X^T
                # chunk); two 128-seq chunks share one PSUM bank and drain
                # with a single DVE copy
                for jp in range(2):
                    vacc = pacc.tile([128, 2, HD], F32, tag="acc")
                    for half in range(2):
                        j = 2 * jp + half
                        for h in range(NHT):
                            xh = (xta if h < NHT // 2 else xtb_)
                            nc.tensor.matmul(vacc[:, half, :],
                                             xh[:, h % (NHT // 2), bass.ts(j, 128)],
                                             wv_t[:, h, :],
                                             start=(h == 0), stop=(h == NHT - 1))
                    nc.vector.tensor_copy(
                        v_sb[:, bass.ts(2 * s + jp, 256)], vacc[:, :, :])

            # ---- phase 2: attention, software-pipelined in PAIRS of key
            # tiles. Scores for a pair land in one 2-bank PSUM tile so a
            # single wide ACT exp covers both tiles ((N+352) fixed overhead
            # amortizes: 574 ns/tile vs 720, below the 645 ns/tile PE rate).
            def attention(qt, ot, s):
                sl = bass.ts(s, SW)
                nk = 4 * s + 4
                pv = ppv.tile([128, SW], F32, tag="pv")
                den = ppv.tile([128, SW], F32, tag="pv")
                exs = {}

                def emit_pair(pi):
                    t0 = 2 * pi
                    stp = pst.tile([128, 2, SW], F32, tag="st")
                    exp_ = xep.tile([128, 2, SW], FP16, tag="ex")
                    vss = []
                    for half, t in enumerate((t0, t0 + 1)):
                        off = t - 4 * s
                        vs = 0 if off < 0 else 128 * off
                        vss.append(vs)
                        nc.tensor.matmul(stp[:, half, vs:],
                                         kt_sb[:, bass.ts(t, 128)],
                                         qt[:, SW * s + vs:SW * (s + 1)],
                                         start=True, stop=True)
                    if t0 + 1 < 4 * s:
                        # both halves full: one wide exp
                        nc.scalar.activation(exp_[:, :, :], stp[:, :, :],
                                             _ACT.Exp, scale=ISQ,
                                             bias=exc_t[:])
                    else:
                        for half, vs in enumerate(vss):
                            nc.scalar.activation(exp_[:, half, vs:],
                                                 stp[:, half, vs:], _ACT.Exp,
                                                 scale=ISQ, bias=exc_t[:])
                            # diagonal 128x128 block needs the triangular mask
                            nc.vector.tensor_tensor(
                                out=exp_[:, half, vs:vs + 128],
                                in0=exp_[:, half, vs:vs + 128],
                                in1=tri_t[:], op=_ALU.mult)
                    exs[pi] = (exp_, vss)

                emit_pair(0)
                for pi in range(nk // 2):
                    if pi + 1 < nk // 2:
                        emit_pair(pi + 1)
                    exp_, vss = exs.pop(pi)
                    for half, vs in enumerate(vss):
                        t = 2 * pi + half
                        st_, sp_ = (t == 0), (t == nk - 1)
                        nc.tensor.matmul(pv[:, vs:], v_sb[:, bass.ts(t, 128)],
                                         exp_[:, half, vs:], start=st_, stop=sp_)
                        nc.tensor.matmul(den[:, vs:], ones_t[:],
                                         exp_[:, half, vs:], start=st_, stop=sp_)
                # 1/den = exp(-ln(den)) on ACT (same table set as the exps)
                lnd = ep.tile([128, SW], F32, tag="lnd")
                nc.scalar.activation(lnd[:], den[:], _ACT.Ln)
                rdn = ep.tile([128, SW], F32, tag="rdn")
                nc.scalar.activation(rdn[:], lnd[:], _ACT.Exp, scale=-1.0)
                nc.vector.tensor_tensor(out=ot[:, sl], in0=pv[:], in1=rdn[:],
                                        op=_ALU.mult)

            # ---- phase 3: output projection (per 128-row tile) ------------
            def outproj(cs):
                _mark(nc, f"C{cs}")
                for m in range(4 * cs, 4 * cs + 4):
                    ob = obp.tile([128, HID], FP16, tag="ob")
                    for np_ in range(2):
                        oup = pst.tile([128, 2, SW], F32, tag="st")
                        for half in range(2):
                            n = 2 * np_ + half
                            nc.tensor.matmul(oup[:, half, :],
                                             ot0[:, bass.ts(m, 128)],
                                             wo_t[:, 0, bass.ts(n, SW)],
                                             start=True, stop=False)
                            nc.tensor.matmul(oup[:, half, :],
                                             ot1[:, bass.ts(m, 128)],
                                             wo_t[:, 1, bass.ts(n, SW)],
                                             start=False, stop=True)
                        # wide pair copies on DVE; ACT stays clear for the
                        # interleaved attention exps
                        nc.vector.tensor_copy(ob[:, bass.ts(np_, 2 * SW)],
                                              oup[:, :, :])
                    nc.sync.dma_start(out[bass.ts(m, 128), :], ob[:])

            # schedule: A0 A1 B(s0) A2 B(s1) C0 A3 B(s2) C1 B(s3) C2 C3
            # (attention for strip s slots in right after A(s+1), keeping PE
            # dense across phase boundaries and the HAM clock warm)
            def phase_b(s):
                _mark(nc, f"B0s{s}")
                attention(qt0, ot0, s)
                _mark(nc, f"B1s{s}")
                attention(qt1, ot1, s)

            phase_a(0)
            phase_a(1)
            phase_b(0)
            phase_a(2)
            phase_b(1)
            outproj(0)
            phase_a(3)
            push_chain()
            push_chain()
            phase_b(2)
            outproj(1)
            phase_b(3)
            outproj(2)
            outproj(3)

    if legalize:
        legalize_waits(nc)
    return nc


# ---------------------------------------------------------------------------
# Host-side input prep.
def _rope_tables(position_ids: np.ndarray):
    pos = position_ids.reshape(-1).astype(np.float64)  # [S]
    j = np.arange(0, HD, 2, dtype=np.float64)
    inv_freq = 1.0 / (THETA ** (j / HD))               # [HD/2]
    freqs = np.outer(inv_freq, pos)                    # [HD/2, S]
    cos_h = np.cos(freqs)
    sin_h = np.sin(freqs)
    cosT = np.concatenate([cos_h, cos_h], axis=0).astype(np.float32)
    sinN = np.concatenate([sin_h, -sin_h], axis=0).astype(np.float32)
    return np.ascontiguousarray(cosT), np.ascontiguousarray(sinN)


def _prep_in_maps(hidden_states, Wq, Wk, Wv, Wo, q_norm_w, k_norm_w,
                  position_ids):
    X = np.asarray(hidden_states, dtype=np.float32).reshape(S, HID)
    # xtb[p, s, h, w] = X[s*SW+w, h*128+p]
    xtb = np.ascontiguousarray(
        X.reshape(NSTRIP, SW, NHT, 128).transpose(3, 0, 2, 1)
        .reshape(128, NSTRIP * NHT * SW)).astype(np.float16)
    cosT, sinN = _rope_tables(np.asarray(position_ids))
    qw = np.asarray(q_norm_w, dtype=np.float32).reshape(HD, 1)
    kw = np.asarray(k_norm_w, dtype=np.float32).reshape(HD, 1)
    cosq = np.ascontiguousarray(cosT * qw).astype(np.float16)
    sinq = np.ascontiguousarray(sinN * qw).astype(np.float16)
    cosk = np.ascontiguousarray(cosT * kw).astype(np.float16)
    sink = np.ascontiguousarray(sinN * kw).astype(np.float16)
    kp, qp = np.meshgrid(np.arange(128), np.arange(128), indexing="ij")
    trimask = (qp >= kp).astype(np.float16)
    onesm = np.ones((128, 128), np.float16)

    Wq = np.asarray(Wq, dtype=np.float32)
    Wk = np.asarray(Wk, dtype=np.float32)
    Wv = np.asarray(Wv, dtype=np.float32)
    Wo = np.asarray(Wo, dtype=np.float32)

    in_maps = []
    for c in range(NCORES):
        kv = c // (NCORES // NKV)
        wq_c = Wq[:, c * HPC * HD:(c + 1) * HPC * HD]
        wq_l = np.ascontiguousarray(
            wq_c.reshape(NHT, 128, HPC * HD).transpose(1, 0, 2).reshape(
                128, NHT * HPC * HD)).astype(np.float16)
        wk_c = Wk[:, kv * HD:(kv + 1) * HD]
        wk_l = np.ascontiguousarray(
            wk_c.reshape(NHT, 128, HD).transpose(1, 0, 2).reshape(
                128, NHT * HD)).astype(np.float16)
        wv_c = Wv[:, kv * HD:(kv + 1) * HD]
        wv_l = np.ascontiguousarray(
            wv_c.reshape(NHT, 128, HD).transpose(1, 0, 2).reshape(
                128, NHT * HD)).astype(np.float16)
        wo_c = Wo[c * HPC * HD:(c + 1) * HPC * HD, :]
        wo_l = np.ascontiguousarray(
            wo_c.reshape(HPC, HD, HID).transpose(1, 0, 2).reshape(
                128, HPC * HID)).astype(np.float16)
        in_maps.append({
            "xtb": xtb, "wqb": wq_l, "wkb": wk_l, "wvb": wv_l, "wob": wo_l,
            "cosq": cosq, "sinq": sinq, "cosk": cosk, "sink": sink,
            "trimask": trimask, "onesm": onesm,
            "epsb": np.full((HD, 1), EPS, np.float32),
            "expcb": np.full((HD, 1), -EXPC, np.float32),
        })
    return in_maps


# ---------------------------------------------------------------------------
# Runner: persistent jitted shard_map over 8 cores (no donation so device
# buffers are reusable across timing iterations).
_CACHE: dict = {}


def _make_runner(nc):
    import jax
    from jax.sharding import Mesh, PartitionSpec
    try:
        from jax.experimental.shard_map import shard_map
    except ImportError:
        from jax.shard_map import shard_map
    from concourse.bass2jax import (_bass_exec_p, install_neuronx_cc_hook,
                                    partition_id_tensor)

    install_neuronx_cc_hook()

    partition_name = (nc.partition_id_tensor.name
                      if nc.partition_id_tensor else None)
    in_names, out_names, out_avals, zero_outs = [], [], [], []
    for alloc in nc.m.functions[0].allocations:
        if not isinstance(alloc, mybir.MemoryLocationSet):
            continue
        name = alloc.memorylocations[0].name
        if alloc.kind == "ExternalInput":
            if name != partition_name:
                in_names.append(name)
        elif alloc.kind == "ExternalOutput":
            shape = list(alloc.tensor_shape)
            npdt = mybir.dt.np(alloc.dtype)
            out_names.append(name)
            out_avals.append(jax.core.ShapedArray(shape, npdt))
            zero_outs.append(np.zeros(shape, npdt))

    n_params = len(in_names)
    all_in_names = list(in_names) + list(out_names)
    if partition_name is not None:
        all_in_names.append(partition_name)

    def _body(*args):
        operands = list(args)
        if partition_name is not None:
            operands.append(partition_id_tensor())
        outs = _bass_exec_p.bind(
            *operands,
            out_avals=tuple(out_avals),
            in_names=tuple(all_in_names),
            out_names=tuple(out_names),
            lowering_input_output_aliases=(),
            sim_require_finite=True,
            sim_require_nnan=True,
            nc=nc,
        )
        return tuple(outs)

    devices = jax.devices()[:NCORES]
    mesh = Mesh(np.asarray(devices), ("core",))
    n_outs = len(out_names)
    sharded = jax.jit(
        shard_map(_body, mesh=mesh,
                  in_specs=(PartitionSpec("core"),) * (n_params + n_outs),
                  out_specs=(PartitionSpec("core"),) * n_outs,
                  check_rep=False),
        keep_unused=True,
    )
    return {
        "fn": sharded, "in_names": in_names, "out_names": out_names,
        "out_avals": out_avals, "zero_outs": zero_outs, "jax": jax,
    }


def _get_runner(which="main"):
    key = f"runner_{which}"
    if key not in _CACHE:
        nc = build_nc() if which == "main" else build_null_nc()
        _CACHE[f"nc_{which}"] = nc
        _CACHE[key] = _make_runner(nc)
    return _CACHE[key]


def _device_args(in_maps, which="main"):
    r = _get_runner(which)
    jax = r["jax"]
    concat_in = [
        np.concatenate([np.asarray(in_maps[c][name]) for c in range(NCORES)],
                       axis=0)
        for name in r["in_names"]
    ]
    concat_zeros = [
        np.zeros((NCORES * z.shape[0], *z.shape[1:]), z.dtype)
        for z in r["zero_outs"]
    ]
    return [jax.device_put(a) for a in (concat_in + concat_zeros)]


def _run(dargs, which="main"):
    r = _get_runner(which)
    outs = r["fn"](*dargs)
    return outs


def kernel(**inputs) -> np.ndarray:
    in_maps = _prep_in_maps(**inputs)
    dargs = _device_args(in_maps)
    outs = _run(dargs)
    out_c = np.asarray(outs[0]).reshape(NCORES, S, HID)
    full = out_c.astype(np.float32).sum(axis=0).astype(np.float32)
    return full.reshape(B, S, HID)


def build_null_nc(legalize=True):
    """Input-identical null kernel: same ExternalInput/Output set, but only a
    trivial copy. Used to calibrate away per-dispatch input-staging overhead
    when estimating device execution time."""
    nc = bass.Bass()
    tensors = [
        ("xtb", [128, NSTRIP * NHT * SW], FP16),
        ("wqb", [128, NHT * HPC * HD], FP16),
        ("wkb", [128, NHT * HD], FP16), ("wvb", [128, NHT * HD], FP16),
        ("wob", [128, HPC * HID], FP16), ("cosq", [HD, S], FP16),
        ("sinq", [HD, S], FP16), ("cosk", [HD, S], FP16),
        ("sink", [HD, S], FP16),
        ("trimask", [128, 128], FP16), ("onesm", [128, 128], FP16),
        ("epsb", [HD, 1], F32), ("expcb", [HD, 1], F32),
    ]
    handles = {}
    for name, shape, dt in tensors:
        handles[name] = nc.dram_tensor(name, shape, dt, kind="ExternalInput")
    out = nc.dram_tensor("out", [S, HID], FP16, kind="ExternalOutput")
    with tile.TileContext(nc) as tc:
        with tc.tile_pool(name="sb", bufs=1) as sb:
            t = sb.tile([128, 128], FP16)
            nc.sync.dma_start(t[:], handles["trimask"][:])
            nc.sync.dma_start(out[0:128, 0:128], t[:])
    if legalize:
        legalize_waits(nc)
    return nc


def timed_run(inputs, iters=60):
    """Estimate on-device execution time.

    Per-call wall time through the axon tunnel is dominated by input staging
    (~30 ms for this input set), so we interleave single calls of the real
    kernel and an input-identical null kernel and difference the medians of
    the paired per-call times."""
    import time
    in_maps = _prep_in_maps(**inputs)
    d_main = _device_args(in_maps, "main")
    d_null = _device_args(in_maps, "null")
    r_main = _get_runner("main")
    r_null = _get_runner("null")
    jax = r_main["jax"]
    jax.block_until_ready(_run(d_main, "main"))
    jax.block_until_ready(_run(d_null, "null"))

    tm, tn = [], []
    for _ in range(iters):
        t0 = time.perf_counter()
        jax.block_until_ready(_run(d_null, "null"))
        tn.append(time.perf_counter() - t0)
        t0 = time.perf_counter()
        jax.block_until_ready(_run(d_main, "main"))
        tm.append(time.perf_counter() - t0)
    tm, tn = np.array(tm), np.array(tn)
    est = float(np.median(tm) - np.median(tn))
    return max(est, 0.0), float(np.median(tm)), float(np.median(tn))
